# revision 8
# baseline (speedup 1.0000x reference)
"""Batched GNN neighbor aggregation on 8 NeuronCores.

out[b] = neibors[b] @ last_embs[b]  for b in 0..7  (2048x2048 @ 2048x128, f32)

Sharding: one graph per core (batch dim across the 8 cores), no cross-core
communication.

Device-side math: the PE contracts over the partition dimension, so the
adjacency operand must sit in SBUF with the contraction index (m) on
partitions; we pre-transpose each graph's adjacency on the host during
sharding so the device streams it with fully-contiguous 1MB DMAs.

Precision: TRN2's native fp32 matmul runs LOW/HIGH two-pass per operand
pair (~19 TF/s), which is slower than the HBM stream. Instead each f32
operand is split on the host into bf16 hi + bf16 lo (hi = RNE-rounded
bf16, lo = bf16 of the exact residual), and the device computes
  out = Ah@Eh + Ah@El + Al@Eh
in three bf16 passes accumulated in fp32 PSUM. The dropped Al@El term is
O(2^-18) relative, so the result keeps fp32-class accuracy while moving
the same 4 bytes/element over HBM and running the PE at bf16 rate.

The device computes out^T = embs^T @ neibors^T with the embedding K-chunks
stationary; the host transposes the (small) result back.
"""

import numpy as np
import ml_dtypes

BF16 = ml_dtypes.bfloat16

B = 8      # graphs == cores
N = 2048   # nodes per graph
D = 128    # embedding size
KT = 128   # contraction tile (partition dim)
NT = 512   # node tile (one f32 PSUM bank)
NK = N // KT  # 16
NN = N // NT  # 4

_cached_nc = None


def _build_program():
    import concourse.tile as tile
    from concourse import bacc, mybir

    f32 = mybir.dt.float32
    bf16 = mybir.dt.bfloat16
    nc = bacc.Bacc(
        "TRN2",
        target_bir_lowering=False,
        debug=False,
        enable_asserts=False,
        enable_partition_id=False,
    )

    # a2[k, s, p, n] = bf16 half s (0=hi, 1=lo) of neibors[g].T[k*128 + p, n]
    a2 = nc.dram_tensor("a2", [NK, 2, KT, N], bf16, kind="ExternalInput")
    # e2[s, p, k, d] = bf16 half s of last_embs[g][k*128 + p, d]
    e2 = nc.dram_tensor("e2", [2, KT, NK, D], bf16, kind="ExternalInput")
    # out_t[d, n] = out[g][n, d]
    out_t = nc.dram_tensor("out_t", [D, N], f32, kind="ExternalOutput")

    # (e_half, a_half) product passes; Al@El is dropped (O(2^-18)).
    # Eh-weighted passes adjacent so the stationary operand changes only
    # twice per strip.
    PASSES = [(0, 0), (0, 1), (1, 0)]

    with tile.TileContext(nc) as tc:
        with (
            tc.tile_pool(name="econst", bufs=1) as epool,
            tc.tile_pool(name="at", bufs=6) as apool,
            tc.tile_pool(name="psum", bufs=1, space="PSUM") as pspool,
            tc.tile_pool(name="out", bufs=1) as opool,
        ):
            e2_r = e2.ap().rearrange("s p k d -> p s k d")
            e_sb = epool.tile([KT, 2, NK, D], bf16)
            # k=0 chunk of Eh first so the first matmul can start ASAP
            nc.sync.dma_start(e_sb[:, 0, 0], e2_r[:, 0, 0])
            nc.scalar.dma_start(e_sb[:, 1], e2_r[:, 1])
            nc.sync.dma_start(e_sb[:, 0, 1:], e2_r[:, 0, 1:])

            ps = [
                pspool.tile([D, NT], f32, name=f"ps{n}", tag=f"ps{n}")
                for n in range(NN)
            ]

            for k in range(NK):
                strip = apool.tile([KT, 2, N], bf16, tag="strip")
                # hi half on the SP ring, lo half on the ACT ring: per-DMA
                # fixed costs overlap but strips still complete in order
                a2_r = a2.ap()[k].rearrange("s p n -> p s n")
                if k == 0:
                    # fine-grained so the PE can start on the first n-chunk
                    # while the rest is still in flight
                    for n in range(NN):
                        nc.sync.dma_start(
                            strip[:, 0, n * NT : (n + 1) * NT],
                            a2_r[:, 0, n * NT : (n + 1) * NT],
                        )
                    nc.scalar.dma_start(strip[:, 1], a2_r[:, 1])
                else:
                    nc.sync.dma_start(strip[:, 0], a2_r[:, 0])
                    nc.scalar.dma_start(strip[:, 1], a2_r[:, 1])
                if k < NK - 1:
                    for pi, (se, sa) in enumerate(PASSES):
                        for n in range(NN):
                            nc.tensor.matmul(
                                ps[n][:],
                                e_sb[:, se, k, :],
                                strip[:, sa, n * NT : (n + 1) * NT],
                                start=(k == 0 and pi == 0),
                                stop=False,
                            )
                else:
                    # last strip: bank-major so each bank's store can start
                    # while the remaining banks are still accumulating
                    for n in range(NN):
                        for pi, (se, sa) in enumerate(PASSES):
                            nc.tensor.matmul(
                                ps[n][:],
                                e_sb[:, se, k, :],
                                strip[:, sa, n * NT : (n + 1) * NT],
                                start=False,
                                stop=(pi == len(PASSES) - 1),
                            )
                        o_sb = opool.tile(
                            [D, NT], f32, name=f"o{n}", tag=f"o{n}"
                        )
                        nc.vector.tensor_copy(o_sb[:], ps[n][:])
                        (nc.sync if n % 2 == 0 else nc.scalar).dma_start(
                            out_t.ap()[:, n * NT : (n + 1) * NT], o_sb[:]
                        )

    nc.compile()
    return nc


def _split_hi_lo(x):
    """Exact-residual bf16 split: x ~= hi + lo with |x - hi - lo| <= 2^-18|x|."""
    hi = x.astype(BF16)
    lo = (x - hi.astype(np.float32)).astype(BF16)
    return hi, lo


def _make_in_maps(last_embs, neibors):
    in_maps = []
    for g in range(B):
        at_g = np.ascontiguousarray(neibors[g].T)  # [m, n]
        ah, al = _split_hi_lo(at_g)
        a2 = np.stack(
            [ah.reshape(NK, KT, N), al.reshape(NK, KT, N)], axis=1
        )  # [NK, 2, KT, N]
        eh, el = _split_hi_lo(np.ascontiguousarray(last_embs[g]))  # [N, D]
        e2 = np.stack(
            [eh.reshape(NK, KT, D), el.reshape(NK, KT, D)], axis=0
        )  # [2, NK, KT, D]
        e2 = np.ascontiguousarray(e2.transpose(0, 2, 1, 3))  # [2, KT, NK, D]
        in_maps.append({"a2": np.ascontiguousarray(a2), "e2": e2})
    return in_maps


def _gather(results):
    out = np.stack([results[g]["out_t"].T for g in range(B)], axis=0)
    return np.ascontiguousarray(out).astype(np.float32, copy=False)


def kernel(last_embs, neibors):
    global _cached_nc
    from concourse.bass_utils import run_bass_kernel_spmd

    last_embs = np.asarray(last_embs, dtype=np.float32)
    neibors = np.asarray(neibors, dtype=np.float32)
    if _cached_nc is None:
        _cached_nc = _build_program()
    res = run_bass_kernel_spmd(
        _cached_nc, _make_in_maps(last_embs, neibors), list(range(B))
    ).results
    return _gather(res)


# revision 9
# speedup vs baseline: 1.0282x; 1.0282x over previous
"""Batched GNN neighbor aggregation on 8 NeuronCores.

out[b] = neibors[b] @ last_embs[b]  for b in 0..7  (2048x2048 @ 2048x128, f32)

Sharding: one graph per core (batch dim across the 8 cores), no cross-core
communication.

Device-side math: the PE contracts over the partition dimension, so the
adjacency operand must sit in SBUF with the contraction index (m) on
partitions; we pre-transpose each graph's adjacency on the host during
sharding so the device streams it with fully-contiguous 1MB DMAs.

Precision: TRN2's native fp32 matmul runs LOW/HIGH two-pass per operand
pair (~19 TF/s), which is slower than the HBM stream. Instead each f32
operand is split on the host into bf16 hi + bf16 lo (hi = RNE-rounded
bf16, lo = bf16 of the exact residual), and the device computes
  out = Ah@Eh + Ah@El + Al@Eh
in three bf16 passes accumulated in fp32 PSUM. The dropped Al@El term is
O(2^-18) relative, so the result keeps fp32-class accuracy while moving
the same 4 bytes/element over HBM and running the PE at bf16 rate.

The device computes out^T = embs^T @ neibors^T with the embedding K-chunks
stationary; the host transposes the (small) result back.
"""

import numpy as np
import ml_dtypes

BF16 = ml_dtypes.bfloat16

B = 8      # graphs == cores
N = 2048   # nodes per graph
D = 128    # embedding size
KT = 128   # contraction tile (partition dim)
NT = 512   # node tile (one f32 PSUM bank)
NK = N // KT  # 16
NN = N // NT  # 4

_cached_nc = None


def _build_program():
    import concourse.tile as tile
    from concourse import bacc, mybir

    f32 = mybir.dt.float32
    bf16 = mybir.dt.bfloat16
    nc = bacc.Bacc(
        "TRN2",
        target_bir_lowering=False,
        debug=False,
        enable_asserts=False,
        enable_partition_id=False,
    )

    # a2[k, s, p, n] = bf16 half s (0=hi, 1=lo) of neibors[g].T[k*128 + p, n]
    a2 = nc.dram_tensor("a2", [NK, 2, KT, N], bf16, kind="ExternalInput")
    # e2[s, p, k, d] = bf16 half s of last_embs[g][k*128 + p, d]
    e2 = nc.dram_tensor("e2", [2, KT, NK, D], bf16, kind="ExternalInput")
    # out_t[d, n] = out[g][n, d]
    out_t = nc.dram_tensor("out_t", [D, N], f32, kind="ExternalOutput")

    # (e_half, a_half) product passes; Al@El is dropped (O(2^-18)).
    # Eh-weighted passes adjacent so the stationary operand changes only
    # twice per strip.
    PASSES = [(0, 0), (0, 1), (1, 0)]

    with tile.TileContext(nc) as tc:
        with (
            tc.tile_pool(name="econst", bufs=1) as epool,
            tc.tile_pool(name="at", bufs=6) as apool,
            tc.tile_pool(name="psum", bufs=1, space="PSUM") as pspool,
            tc.tile_pool(name="out", bufs=1) as opool,
        ):
            e2_r = e2.ap().rearrange("s p k d -> p s k d")
            e_sb = epool.tile([KT, 2, NK, D], bf16)
            # k=0 chunk of Eh first so the first matmul can start ASAP
            nc.sync.dma_start(e_sb[:, 0, 0], e2_r[:, 0, 0])
            nc.scalar.dma_start(e_sb[:, 1], e2_r[:, 1])
            nc.sync.dma_start(e_sb[:, 0, 1:], e2_r[:, 0, 1:])

            ps = [
                pspool.tile([D, NT], f32, name=f"ps{n}", tag=f"ps{n}")
                for n in range(NN)
            ]

            for k in range(NK):
                strip = apool.tile([KT, 2, N], bf16, tag="strip")
                # hi half on the SP ring, lo half on the ACT ring: per-DMA
                # fixed costs overlap but strips still complete in order
                a2_r = a2.ap()[k].rearrange("s p n -> p s n")
                if k == 0:
                    # fine-grained so the PE can start on the first n-chunk
                    # while the rest is still in flight
                    for n in range(NN):
                        nc.sync.dma_start(
                            strip[:, 0, n * NT : (n + 1) * NT],
                            a2_r[:, 0, n * NT : (n + 1) * NT],
                        )
                    nc.scalar.dma_start(strip[:, 1], a2_r[:, 1])
                else:
                    nc.sync.dma_start(strip[:, 0], a2_r[:, 0])
                    nc.scalar.dma_start(strip[:, 1], a2_r[:, 1])
                if k < NK - 1:
                    for pi, (se, sa) in enumerate(PASSES):
                        for n in range(NN):
                            nc.tensor.matmul(
                                ps[n][:],
                                e_sb[:, se, k, :],
                                strip[:, sa, n * NT : (n + 1) * NT],
                                start=(k == 0 and pi == 0),
                                stop=False,
                            )
                else:
                    # last strip: bank-major so each bank's store can start
                    # while the remaining banks are still accumulating
                    for n in range(NN):
                        for pi, (se, sa) in enumerate(PASSES):
                            nc.tensor.matmul(
                                ps[n][:],
                                e_sb[:, se, k, :],
                                strip[:, sa, n * NT : (n + 1) * NT],
                                start=False,
                                stop=(pi == len(PASSES) - 1),
                            )
                        o_sb = opool.tile(
                            [D, NT], f32, name=f"o{n}", tag=f"o{n}"
                        )
                        nc.vector.tensor_copy(o_sb[:], ps[n][:])
                        (nc.sync if n % 2 == 0 else nc.scalar).dma_start(
                            out_t.ap()[:, n * NT : (n + 1) * NT], o_sb[:]
                        )

    _dedup_ldweights(nc, mybir)
    nc.compile()
    return nc


def _dedup_ldweights(nc, mybir):
    """Drop InstLdweights whose weight AP matches the immediately preceding
    weight load in the PE stream (matmuls here have ldweights=False, so the
    stationary operand stays in the array between identical loads)."""
    for bb in nc.m.functions[0].blocks:
        insts = bb.instructions
        last_key = None
        removed = []
        for inst in insts:
            if getattr(inst, "engine", None) != mybir.EngineType.PE:
                continue
            ty = type(inst).__name__
            if ty == "InstLdweights":
                key = repr(inst.ins[0])
                if key == last_key and not inst.has_wait():
                    removed.append(inst)
                else:
                    last_key = key
            elif ty != "InstMatmult":
                last_key = None
        if removed:
            rm = {id(i) for i in removed}
            insts[:] = [i for i in insts if id(i) not in rm]
            for i in removed:
                nc.inst_map.pop(i.name, None)


def _split_hi_lo(x):
    """Exact-residual bf16 split: x ~= hi + lo with |x - hi - lo| <= 2^-18|x|."""
    hi = x.astype(BF16)
    lo = (x - hi.astype(np.float32)).astype(BF16)
    return hi, lo


def _make_in_maps(last_embs, neibors):
    in_maps = []
    for g in range(B):
        at_g = np.ascontiguousarray(neibors[g].T)  # [m, n]
        ah, al = _split_hi_lo(at_g)
        a2 = np.stack(
            [ah.reshape(NK, KT, N), al.reshape(NK, KT, N)], axis=1
        )  # [NK, 2, KT, N]
        eh, el = _split_hi_lo(np.ascontiguousarray(last_embs[g]))  # [N, D]
        e2 = np.stack(
            [eh.reshape(NK, KT, D), el.reshape(NK, KT, D)], axis=0
        )  # [2, NK, KT, D]
        e2 = np.ascontiguousarray(e2.transpose(0, 2, 1, 3))  # [2, KT, NK, D]
        in_maps.append({"a2": np.ascontiguousarray(a2), "e2": e2})
    return in_maps


def _gather(results):
    out = np.stack([results[g]["out_t"].T for g in range(B)], axis=0)
    return np.ascontiguousarray(out).astype(np.float32, copy=False)


def kernel(last_embs, neibors):
    global _cached_nc
    from concourse.bass_utils import run_bass_kernel_spmd

    last_embs = np.asarray(last_embs, dtype=np.float32)
    neibors = np.asarray(neibors, dtype=np.float32)
    if _cached_nc is None:
        _cached_nc = _build_program()
    res = run_bass_kernel_spmd(
        _cached_nc, _make_in_maps(last_embs, neibors), list(range(B))
    ).results
    return _gather(res)


# revision 11
# speedup vs baseline: 1.0510x; 1.0222x over previous
"""Batched GNN neighbor aggregation on 8 NeuronCores.

out[b] = neibors[b] @ last_embs[b]  for b in 0..7  (2048x2048 @ 2048x128, f32)

Sharding: one graph per core (batch dim across the 8 cores), no cross-core
communication.

Device-side math: the PE contracts over the partition dimension, so the
adjacency operand must sit in SBUF with the contraction index (m) on
partitions; we pre-transpose each graph's adjacency on the host during
sharding so the device streams it with fully-contiguous 1MB DMAs.

Precision: TRN2's native fp32 matmul runs LOW/HIGH two-pass per operand
pair (~19 TF/s), which is slower than the HBM stream. Instead each f32
operand is split on the host into bf16 hi + bf16 lo (hi = RNE-rounded
bf16, lo = bf16 of the exact residual), and the device computes
  out = Ah@Eh + Ah@El + Al@Eh
in three bf16 passes accumulated in fp32 PSUM. The dropped Al@El term is
O(2^-18) relative, so the result keeps fp32-class accuracy while moving
the same 4 bytes/element over HBM and running the PE at bf16 rate.

The device computes out^T = embs^T @ neibors^T with the embedding K-chunks
stationary; the host transposes the (small) result back.
"""

import numpy as np
import ml_dtypes

BF16 = ml_dtypes.bfloat16

B = 8      # graphs == cores
N = 2048   # nodes per graph
D = 128    # embedding size
KT = 128   # contraction tile (partition dim)
NT = 512   # node tile (one f32 PSUM bank)
NK = N // KT  # 16
NN = N // NT  # 4

_cached_nc = None


def _build_program():
    import concourse.tile as tile
    from concourse import bacc, mybir

    f32 = mybir.dt.float32
    bf16 = mybir.dt.bfloat16
    nc = bacc.Bacc(
        "TRN2",
        target_bir_lowering=False,
        debug=False,
        enable_asserts=False,
        enable_partition_id=False,
    )

    # a2[k, s, p, n] = bf16 half s (0=hi, 1=lo) of neibors[g].T[k*128 + p, n]
    a2 = nc.dram_tensor("a2", [NK, 2, KT, N], bf16, kind="ExternalInput")
    # e2[s, p, k, d] = bf16 half s of last_embs[g][k*128 + p, d]
    e2 = nc.dram_tensor("e2", [2, KT, NK, D], bf16, kind="ExternalInput")
    # out_t[d, n] = out[g][n, d]
    out_t = nc.dram_tensor("out_t", [D, N], f32, kind="ExternalOutput")

    # (e_half, a_half) product passes; Al@El is dropped (O(2^-18)).
    # Eh-weighted passes adjacent so the stationary operand changes only
    # twice per strip.
    PASSES = [(0, 0), (0, 1), (1, 0)]

    with tile.TileContext(nc) as tc:
        with (
            tc.tile_pool(name="econst", bufs=1) as epool,
            tc.tile_pool(name="at", bufs=8) as apool,
            tc.tile_pool(name="psum", bufs=1, space="PSUM") as pspool,
            tc.tile_pool(name="out", bufs=1) as opool,
        ):
            e2_r = e2.ap().rearrange("s p k d -> p s k d")
            e_sb = epool.tile([KT, 2, NK, D], bf16)
            # k=0 chunk of Eh first so the first matmul can start ASAP
            nc.sync.dma_start(e_sb[:, 0, 0], e2_r[:, 0, 0])
            nc.scalar.dma_start(e_sb[:, 1], e2_r[:, 1])
            nc.sync.dma_start(e_sb[:, 0, 1:], e2_r[:, 0, 1:])

            ps = [
                pspool.tile([D, NT], f32, name=f"ps{n}", tag=f"ps{n}")
                for n in range(NN)
            ]

            for k in range(NK):
                strip = apool.tile([KT, 2, N], bf16, tag="strip")
                # hi half on the SP ring, lo half on the ACT ring: per-DMA
                # fixed costs overlap but strips still complete in order
                a2_r = a2.ap()[k].rearrange("s p n -> p s n")
                if k == 0:
                    # fine-grained so the PE can start on the first n-chunk
                    # while the rest is still in flight
                    for n in range(NN):
                        nc.sync.dma_start(
                            strip[:, 0, n * NT : (n + 1) * NT],
                            a2_r[:, 0, n * NT : (n + 1) * NT],
                        )
                    nc.scalar.dma_start(strip[:, 1], a2_r[:, 1])
                elif k == NK - 1:
                    # fine-grained per n-chunk (hi+lo together) so each
                    # bank's final accumulation + store pipelines with the
                    # remaining chunks' arrivals
                    for n in range(NN):
                        nc.sync.dma_start(
                            strip[:, :, n * NT : (n + 1) * NT],
                            a2_r[:, :, n * NT : (n + 1) * NT],
                        )
                else:
                    nc.sync.dma_start(strip[:, 0], a2_r[:, 0])
                    nc.scalar.dma_start(strip[:, 1], a2_r[:, 1])
                if k < NK - 1:
                    for pi, (se, sa) in enumerate(PASSES):
                        for n in range(NN):
                            nc.tensor.matmul(
                                ps[n][:],
                                e_sb[:, se, k, :],
                                strip[:, sa, n * NT : (n + 1) * NT],
                                start=(k == 0 and pi == 0),
                                stop=False,
                            )
                else:
                    # last strip: bank-major so each bank's store can start
                    # while the remaining banks are still accumulating
                    for n in range(NN):
                        for pi, (se, sa) in enumerate(PASSES):
                            nc.tensor.matmul(
                                ps[n][:],
                                e_sb[:, se, k, :],
                                strip[:, sa, n * NT : (n + 1) * NT],
                                start=False,
                                stop=(pi == len(PASSES) - 1),
                            )
                        o_sb = opool.tile(
                            [D, NT], f32, name=f"o{n}", tag=f"o{n}"
                        )
                        nc.vector.tensor_copy(o_sb[:], ps[n][:])
                        (nc.sync if n % 2 == 0 else nc.scalar).dma_start(
                            out_t.ap()[:, n * NT : (n + 1) * NT], o_sb[:]
                        )

    _dedup_ldweights(nc, mybir)
    nc.compile()
    return nc


def _dedup_ldweights(nc, mybir):
    """Drop InstLdweights whose weight AP matches the immediately preceding
    weight load in the PE stream (matmuls here have ldweights=False, so the
    stationary operand stays in the array between identical loads)."""
    for bb in nc.m.functions[0].blocks:
        insts = bb.instructions
        last_key = None
        removed = []
        for inst in insts:
            if getattr(inst, "engine", None) != mybir.EngineType.PE:
                continue
            ty = type(inst).__name__
            if ty == "InstLdweights":
                key = repr(inst.ins[0])
                if key == last_key and not inst.has_wait():
                    removed.append(inst)
                else:
                    last_key = key
            elif ty != "InstMatmult":
                last_key = None
        if removed:
            rm = {id(i) for i in removed}
            insts[:] = [i for i in insts if id(i) not in rm]
            for i in removed:
                nc.inst_map.pop(i.name, None)


def _split_hi_lo(x):
    """Exact-residual bf16 split: x ~= hi + lo with |x - hi - lo| <= 2^-18|x|."""
    hi = x.astype(BF16)
    lo = (x - hi.astype(np.float32)).astype(BF16)
    return hi, lo


def _make_in_maps(last_embs, neibors):
    in_maps = []
    for g in range(B):
        at_g = np.ascontiguousarray(neibors[g].T)  # [m, n]
        ah, al = _split_hi_lo(at_g)
        a2 = np.stack(
            [ah.reshape(NK, KT, N), al.reshape(NK, KT, N)], axis=1
        )  # [NK, 2, KT, N]
        eh, el = _split_hi_lo(np.ascontiguousarray(last_embs[g]))  # [N, D]
        e2 = np.stack(
            [eh.reshape(NK, KT, D), el.reshape(NK, KT, D)], axis=0
        )  # [2, NK, KT, D]
        e2 = np.ascontiguousarray(e2.transpose(0, 2, 1, 3))  # [2, KT, NK, D]
        in_maps.append({"a2": np.ascontiguousarray(a2), "e2": e2})
    return in_maps


def _gather(results):
    out = np.stack([results[g]["out_t"].T for g in range(B)], axis=0)
    return np.ascontiguousarray(out).astype(np.float32, copy=False)


def kernel(last_embs, neibors):
    global _cached_nc
    from concourse.bass_utils import run_bass_kernel_spmd

    last_embs = np.asarray(last_embs, dtype=np.float32)
    neibors = np.asarray(neibors, dtype=np.float32)
    if _cached_nc is None:
        _cached_nc = _build_program()
    res = run_bass_kernel_spmd(
        _cached_nc, _make_in_maps(last_embs, neibors), list(range(B))
    ).results
    return _gather(res)


# revision 12
# speedup vs baseline: 1.0774x; 1.0252x over previous
"""Batched GNN neighbor aggregation on 8 NeuronCores.

out[b] = neibors[b] @ last_embs[b]  for b in 0..7  (2048x2048 @ 2048x128, f32)

Sharding: one graph per core (batch dim across the 8 cores), no cross-core
communication.

Device-side math: the PE contracts over the partition dimension, so the
adjacency operand must sit in SBUF with the contraction index (m) on
partitions; we pre-transpose each graph's adjacency on the host during
sharding so the device streams it with fully-contiguous 1MB DMAs.

Precision: TRN2's native fp32 matmul runs LOW/HIGH two-pass per operand
pair (~19 TF/s), which is slower than the HBM stream. Instead each f32
operand is split on the host into bf16 hi + bf16 lo (hi = RNE-rounded
bf16, lo = bf16 of the exact residual), and the device computes
  out = Ah@Eh + Ah@El + Al@Eh
in three bf16 passes accumulated in fp32 PSUM. The dropped Al@El term is
O(2^-18) relative, so the result keeps fp32-class accuracy while moving
the same 4 bytes/element over HBM and running the PE at bf16 rate.

The device computes out^T = embs^T @ neibors^T with the embedding K-chunks
stationary; the host transposes the (small) result back.
"""

import numpy as np
import ml_dtypes

BF16 = ml_dtypes.bfloat16

B = 8      # graphs == cores
N = 2048   # nodes per graph
D = 128    # embedding size
KT = 128   # contraction tile (partition dim)
NT = 512   # node tile (one f32 PSUM bank)
NK = N // KT  # 16
NN = N // NT  # 4

_cached_nc = None


def _build_program():
    import concourse.tile as tile
    from concourse import bacc, mybir

    f32 = mybir.dt.float32
    bf16 = mybir.dt.bfloat16
    nc = bacc.Bacc(
        "TRN2",
        target_bir_lowering=False,
        debug=False,
        enable_asserts=False,
        enable_partition_id=False,
    )

    # a2[k, s, p, n] = bf16 half s (0=hi, 1=lo) of neibors[g].T[k*128 + p, n]
    a2 = nc.dram_tensor("a2", [NK, 2, KT, N], bf16, kind="ExternalInput")
    # e2[s, p, k, d] = bf16 half s of last_embs[g][k*128 + p, d]
    e2 = nc.dram_tensor("e2", [2, KT, NK, D], bf16, kind="ExternalInput")
    # out_t[d, n] = out[g][n, d]
    out_t = nc.dram_tensor("out_t", [D, N], f32, kind="ExternalOutput")

    # (e_half, a_half) product passes; Al@El is dropped (O(2^-18)).
    # Eh-weighted passes adjacent so the stationary operand changes only
    # twice per strip.
    PASSES = [(0, 0), (0, 1), (1, 0)]

    with tile.TileContext(nc) as tc:
        with (
            tc.tile_pool(name="econst", bufs=1) as epool,
            tc.tile_pool(name="at", bufs=8) as apool,
            tc.tile_pool(name="psum", bufs=1, space="PSUM") as pspool,
            tc.tile_pool(name="out", bufs=1) as opool,
        ):
            e2_r = e2.ap().rearrange("s p k d -> p s k d")
            e_sb = epool.tile([KT, 2, NK, D], bf16)
            # k=0 chunk of Eh first so the first matmul can start ASAP
            nc.sync.dma_start(e_sb[:, 0, 0], e2_r[:, 0, 0])
            nc.scalar.dma_start(e_sb[:, 0, 1:], e2_r[:, 0, 1:])
            nc.scalar.dma_start(e_sb[:, 1], e2_r[:, 1])

            ps = [
                pspool.tile([D, NT], f32, name=f"ps{n}", tag=f"ps{n}")
                for n in range(NN)
            ]

            for k in range(NK):
                strip = apool.tile([KT, 2, N], bf16, tag="strip")
                # hi half on the SP ring, lo half on the ACT ring: per-DMA
                # fixed costs overlap but strips still complete in order
                a2_r = a2.ap()[k].rearrange("s p n -> p s n")
                if k == 0:
                    # fine-grained so the PE can start on the first n-chunk
                    # while the rest is still in flight
                    for n in range(NN):
                        nc.sync.dma_start(
                            strip[:, 0, n * NT : (n + 1) * NT],
                            a2_r[:, 0, n * NT : (n + 1) * NT],
                        )
                    nc.scalar.dma_start(strip[:, 1], a2_r[:, 1])
                elif k == NK - 1:
                    # fine-grained per n-chunk (hi+lo together) so each
                    # bank's final accumulation + store pipelines with the
                    # remaining chunks' arrivals
                    for n in range(NN):
                        nc.sync.dma_start(
                            strip[:, :, n * NT : (n + 1) * NT],
                            a2_r[:, :, n * NT : (n + 1) * NT],
                        )
                else:
                    nc.sync.dma_start(strip[:, 0], a2_r[:, 0])
                    nc.scalar.dma_start(strip[:, 1], a2_r[:, 1])
                if k < NK - 1:
                    for pi, (se, sa) in enumerate(PASSES):
                        for n in range(NN):
                            nc.tensor.matmul(
                                ps[n][:],
                                e_sb[:, se, k, :],
                                strip[:, sa, n * NT : (n + 1) * NT],
                                start=(k == 0 and pi == 0),
                                stop=False,
                            )
                else:
                    # last strip: bank-major so each bank's store can start
                    # while the remaining banks are still accumulating
                    for n in range(NN):
                        for pi, (se, sa) in enumerate(PASSES):
                            nc.tensor.matmul(
                                ps[n][:],
                                e_sb[:, se, k, :],
                                strip[:, sa, n * NT : (n + 1) * NT],
                                start=False,
                                stop=(pi == len(PASSES) - 1),
                            )
                        o_sb = opool.tile(
                            [D, NT], f32, name=f"o{n}", tag=f"o{n}"
                        )
                        nc.vector.tensor_copy(o_sb[:], ps[n][:])
                        (nc.sync if n % 2 == 0 else nc.scalar).dma_start(
                            out_t.ap()[:, n * NT : (n + 1) * NT], o_sb[:]
                        )

    _dedup_ldweights(nc, mybir)
    nc.compile()
    return nc


def _dedup_ldweights(nc, mybir):
    """Drop InstLdweights whose weight AP matches the immediately preceding
    weight load in the PE stream (matmuls here have ldweights=False, so the
    stationary operand stays in the array between identical loads)."""
    for bb in nc.m.functions[0].blocks:
        insts = bb.instructions
        last_key = None
        removed = []
        for inst in insts:
            if getattr(inst, "engine", None) != mybir.EngineType.PE:
                continue
            ty = type(inst).__name__
            if ty == "InstLdweights":
                key = repr(inst.ins[0])
                if key == last_key and not inst.has_wait():
                    removed.append(inst)
                else:
                    last_key = key
            elif ty != "InstMatmult":
                last_key = None
        if removed:
            rm = {id(i) for i in removed}
            insts[:] = [i for i in insts if id(i) not in rm]
            for i in removed:
                nc.inst_map.pop(i.name, None)


def _split_hi_lo(x):
    """Exact-residual bf16 split: x ~= hi + lo with |x - hi - lo| <= 2^-18|x|."""
    hi = x.astype(BF16)
    lo = (x - hi.astype(np.float32)).astype(BF16)
    return hi, lo


def _make_in_maps(last_embs, neibors):
    in_maps = []
    for g in range(B):
        at_g = np.ascontiguousarray(neibors[g].T)  # [m, n]
        ah, al = _split_hi_lo(at_g)
        a2 = np.stack(
            [ah.reshape(NK, KT, N), al.reshape(NK, KT, N)], axis=1
        )  # [NK, 2, KT, N]
        eh, el = _split_hi_lo(np.ascontiguousarray(last_embs[g]))  # [N, D]
        e2 = np.stack(
            [eh.reshape(NK, KT, D), el.reshape(NK, KT, D)], axis=0
        )  # [2, NK, KT, D]
        e2 = np.ascontiguousarray(e2.transpose(0, 2, 1, 3))  # [2, KT, NK, D]
        in_maps.append({"a2": np.ascontiguousarray(a2), "e2": e2})
    return in_maps


def _gather(results):
    out = np.stack([results[g]["out_t"].T for g in range(B)], axis=0)
    return np.ascontiguousarray(out).astype(np.float32, copy=False)


def kernel(last_embs, neibors):
    global _cached_nc
    from concourse.bass_utils import run_bass_kernel_spmd

    last_embs = np.asarray(last_embs, dtype=np.float32)
    neibors = np.asarray(neibors, dtype=np.float32)
    if _cached_nc is None:
        _cached_nc = _build_program()
    res = run_bass_kernel_spmd(
        _cached_nc, _make_in_maps(last_embs, neibors), list(range(B))
    ).results
    return _gather(res)
